# revision 3
# baseline (speedup 1.0000x reference)
import functools

import jax
import jax.numpy as jnp
import numpy as np

# nn_CapLayer: grouped 1x1 conv + 3-iter dynamic routing (capsule layer).
#
# Data-parallel over batch: 256 batch elements sharded 32-per-core across
# 8 NeuronCores; conv weight w and bias b_conv replicated. Routing is
# batch-local so there is no cross-device communication.
#
# The routing is computed in FACTORED form: the (bs, 10, 16, 1152) `pred`
# tensor (189 MB fp32 over the full batch) is never materialized. Every
# contraction against pred is pushed through its low-rank structure
# pred = W.xg + bias.
#
# Device-op minimization (per-op dispatch overhead dominates at this
# problem size):
#  * bias folded into the contractions: host prepends a constant-1
#    input-capsule row to x and appends the bias column to W, so the
#    h/cs bias terms ride the same einsums instead of separate kernels.
#  * inputs pre-cast to bf16 on host (input marshalling): heavy
#    contractions run bf16 (4x fp32 PE rate) with fp32 accumulation;
#    squash/softmax stay fp32.
#  * softmax skips max-subtraction (logits are O(1): exp safe in fp32)
#    and defers the 1/sum(e) normalization past the linear contractions:
#    s_r = (e . pred)/Z instead of (e/Z) . pred.

NUM_SHARED = 32
IN_DIM = 8
NUM_OUT = 10
OUT_DIM = 16
ROUTE_NUM = 3
EPS = 1e-20

N_CORES = 8
P = 36
I = NUM_SHARED * P  # 1152 input capsules

BF = jnp.bfloat16
F32 = jnp.float32


def _squash(s):
    # s: (bs, J, D) fp32
    n2 = jnp.sum(s * s, axis=2, keepdims=True)
    n = jnp.sqrt(n2)
    return s * (n2 / (1.0 + n2) / (n + EPS))


def _caps_shard(xgb, Wab):
    # xgb: (bs_l, s, 9, p) bf16  (i''=8 row is the constant-1 bias row)
    # Wab: (s, j, d, 9) bf16     ([..., 8] column is the conv bias)
    # r = 0: c is uniform (softmax of zeros) -> s0 = mean_i pred
    xs0 = jnp.sum(xgb, axis=3, dtype=F32)                   # (b, s, 9)
    s0 = jnp.einsum('bsi,sjdi->bjd', xs0.astype(BF), Wab,
                    preferred_element_type=F32) * (1.0 / I)
    v = _squash(s0)

    L = None  # routing logits, (b, j, s, p) fp32; None means all-zero
    for r in range(1, ROUTE_NUM):
        # b-update: dL[b,j,s,p] = sum_i'' g[b,j,s,i''] * xga[b,s,i'',p]
        # (the i''=8 slot carries the bias term h automatically)
        g = jnp.einsum('bjd,sjdi->bjsi', v.astype(BF), Wab,
                       preferred_element_type=F32).astype(BF)
        dL = jnp.einsum('bjsi,bsip->bjsp', g, xgb,
                        preferred_element_type=F32)         # fp32
        L = dL if L is None else L + dL

        # softmax over i=(s,p), no max-subtraction, normalization deferred
        e = jnp.exp(L)
        Z = jnp.sum(e, axis=(2, 3))                         # (b, j)
        ya = jnp.einsum('bjsp,bsip->bjsi', e.astype(BF), xgb,
                        preferred_element_type=F32)
        s_r = jnp.einsum('bjsi,sjdi->bjd', ya.astype(BF), Wab,
                         preferred_element_type=F32)
        s_r = s_r / Z[:, :, None]
        v = _squash(s_r)
    return v


def _prep(x: np.ndarray, w: np.ndarray, b_conv: np.ndarray):
    """Host-side input marshalling: bias-augmentation + bf16 cast."""
    bs = x.shape[0]
    xg = np.ascontiguousarray(x.reshape(bs, NUM_SHARED, IN_DIM, P))
    xga = np.concatenate(
        [xg, np.ones((bs, NUM_SHARED, 1, P), x.dtype)], axis=2)
    xgb = xga.astype(BF)                                    # (bs, s, 9, p)
    Wr = w.reshape(NUM_SHARED, NUM_OUT, OUT_DIM, IN_DIM)
    Br = b_conv.reshape(NUM_SHARED, NUM_OUT, OUT_DIM)
    Wab = np.concatenate([Wr, Br[..., None]], axis=3).astype(BF)
    return xgb, Wab


@functools.cache
def _pmapped(n_cores: int):
    return jax.pmap(_caps_shard, axis_name='cores', devices=jax.devices()[:n_cores])


@functools.cache
def _pmapped_loop(n_cores: int, n_iter: int):
    # Timing helper: runs the shard computation n_iter times back-to-back
    # on-device inside one dispatch, chaining a data dependency through
    # the input so XLA cannot hoist or CSE the iterations. Device time of
    # one iteration = (t_loop - dispatch_floor) / n_iter.
    def run(xgb, Wab):
        def body(carry, _):
            v = _caps_shard(xgb * (1.0 + carry), Wab)
            return (jnp.max(jnp.abs(v)) * 1e-30).astype(BF), None
        c, _ = jax.lax.scan(body, jnp.bfloat16(0.0), None, length=n_iter)
        return c
    return jax.pmap(run, axis_name='cores', devices=jax.devices()[:n_cores])


def kernel(x: np.ndarray, w: np.ndarray, b_conv: np.ndarray) -> np.ndarray:
    bs = x.shape[0]
    n_cores = N_CORES
    n_dev = len(jax.devices())
    while n_cores > 1 and (n_cores > n_dev or bs % n_cores != 0):
        n_cores //= 2
    shard = bs // n_cores
    xgb, Wab = _prep(x, w, b_conv)
    xs = np.ascontiguousarray(xgb.reshape(n_cores, shard, *xgb.shape[1:]))
    ws = np.ascontiguousarray(np.broadcast_to(Wab, (n_cores,) + Wab.shape))
    v = _pmapped(n_cores)(xs, ws)
    v = np.asarray(v)
    return v.reshape(bs, NUM_OUT, OUT_DIM)
